# revision 1
# baseline (speedup 1.0000x reference)
"""DeltaGradientDescentMemory Trainium2 kernel.

Math: per step t (T=8192, dk=dv=1024, H=8 heads):
    kn_t = k_t / max(||k_t||, eps)
    r_t^h = W_h^T kn_t           (read before update)
    W_h  += kn_t (beta v_t - alpha r_t^h)^T
    out_t = mean_h r_t^h

The recurrence is linear in W0 and the dynamics are head-independent, so
mean_h r^h equals the single recurrence started from S0 = mean_h W0_h.
The dv dimension is fully independent given kn, so the 8 cores shard dv
(128 columns each) with zero cross-core communication.

On-device chunked delta rule (chunk C=128, state Shat = alpha*S in PSUM fp32):
    A  = K K^T                  (Gram, via PE)
    PT = alpha * triu(A, 1)     ( = (alpha L)^T, bf16 )
    Gh = K Shat                 ( = alpha * K S )
    X0 = V - Gh
    U  = X0 - PT.T@(X0 - PT.T@X0)      (2-term Neumann-Horner solve of
                                        (I + alpha L) U = V - alpha*G)
    R  = (Gh + PT.T@U) / alpha  -> output rows
    Shat += (alpha K)^T U       (PSUM accumulate, start=False)
"""
import numpy as np
import ml_dtypes
import orjson

import concourse.bass as bass
import concourse.mybir as mybir
import concourse.tile as tile
from concourse.bass_utils import run_bass_kernel_spmd
from concourse.masks import make_identity, make_upper_triangular

ALPHA = 0.1
EPS = 1e-12
T, DK, DV, H = 8192, 1024, 1024, 8
C = 128                  # chunk length
NCH = T // C             # 64 chunks
NQUAD = NCH // 4         # kT tiles hold 4 chunks each
DVS = DV // 8            # dv shard per core
NJ = DK // 128           # 8 dk tiles
N_DVE_REFRESH = 2        # S-refresh tiles on DVE; rest on ACT

BF16 = mybir.dt.bfloat16
FP32 = mybir.dt.float32

# ---------------------------------------------------------------------------
# Walrus in this container accepts at most ONE sync-wait per instruction, but
# Tile emits several. Legalize the serialized BIR: hoist all but the last wait
# of an instruction onto fresh single-wait EventSemaphore instructions placed
# just before it (same engine stream, order preserved).
_mw_counter = [0]


def _legalize_multiwait_json(bir_bytes: bytes) -> bytes:
    j = orjson.loads(bir_bytes)
    changed = False
    for fn in j.get("functions", []):
        for bb in fn.get("blocks", []):
            out = []
            for inst in bb.get("instructions", []):
                si = inst.get("sync_info")
                waits = si.get("on_wait") if si else None
                if waits and len(waits) > 1:
                    changed = True
                    for w in waits[:-1]:
                        _mw_counter[0] += 1
                        out.append({
                            "debug": inst.get("debug", 0),
                            "engine": inst["engine"],
                            "ins": [],
                            "outs": [],
                            "name": f"mwsplit-{_mw_counter[0]}",
                            "opcode": "EventSemaphore",
                            "sync_info": {"on_update": [], "on_wait": [w]},
                        })
                    si["on_wait"] = waits[-1:]
                out.append(inst)
            bb["instructions"] = out
    return orjson.dumps(j) if changed else bir_bytes


_orig_to_json_bytes = bass.Bass.to_json_bytes


def _patched_to_json_bytes(self):
    return _legalize_multiwait_json(_orig_to_json_bytes(self))


bass.Bass.to_json_bytes = _patched_to_json_bytes


# ---------------------------------------------------------------------------
def build_kernel(n_chunks: int = NCH, reps: int = 1, zero_init: bool = True, use_cross: bool = True) -> bass.Bass:
    nc = bass.Bass(trn_type="TRN2")
    ktt = nc.dram_tensor("ktt", [NQUAD, NJ, 128, 4 * C], BF16, kind="ExternalInput")
    ktl = nc.dram_tensor("ktl", [NCH, C, DK], BF16, kind="ExternalInput")
    vb = nc.dram_tensor("vb", [NCH, C, DVS], BF16, kind="ExternalInput")
    s0 = nc.dram_tensor("s0", [NJ, 128, DVS], FP32, kind="ExternalInput")
    r = nc.dram_tensor("r", [NCH, C, DVS], FP32, kind="ExternalOutput")

    with tile.TileContext(nc) as tc:
        with (
            tc.tile_pool(name="const", bufs=1) as cpool,
            tc.tile_pool(name="kt", bufs=3) as ktpool,
            tc.tile_pool(name="ktl", bufs=3) as ktlpool,
            tc.tile_pool(name="vb", bufs=3) as vbpool,
            tc.tile_pool(name="sb", bufs=1) as sbpool,
            tc.tile_pool(name="wk", bufs=2) as wk,
            tc.tile_pool(name="routq", bufs=2) as routpool,
            tc.tile_pool(name="psS", bufs=1, space="PSUM") as psSp,
            tc.tile_pool(name="psG", bufs=3, space="PSUM") as psGp,
            tc.tile_pool(name="psA", bufs=1, space="PSUM") as psAp,
            tc.tile_pool(name="psX", bufs=2, space="PSUM") as psXp,
        ):
            # constants
            idn = cpool.tile([128, 128], FP32, tag="idn", name="idn")
            make_identity(nc, idn[:])
            mu = cpool.tile([128, 128], BF16, tag="mu", name="mu")
            make_upper_triangular(nc, mu[:], val=ALPHA, diag=False)

            # state init: psS = S0 (via identity matmul, fp32 exact).
            # zero_init: W0 is all zero -> chunk 0/1 state handled by memset
            # of the bf16 snapshots; first Supd uses start=True.
            HALF = NJ // 2 * DVS  # 512
            psS0 = psSp.tile([128, HALF], FP32, tag="psS0", name="psS0")
            psS1 = psSp.tile([128, HALF], FP32, tag="psS1", name="psS1")
            psS_half = [psS0, psS1]
            s0_sb = cpool.tile([128, NJ * DVS], FP32, tag="s0", name="s0sb")
            if zero_init:
                nc.gpsimd.memset(s0_sb[:], 0.0)
            else:
                nc.sync.dma_start(
                    s0_sb[:].rearrange("p (j c) -> p j c", j=NJ),
                    s0[:].rearrange("j p c -> p j c"),
                )
            for j in range(NJ):
                js = slice((j % 4) * DVS, (j % 4 + 1) * DVS)
                nc.tensor.matmul(
                    psS_half[j // 4][:, js], idn[:], s0_sb[:, j * DVS:(j + 1) * DVS],
                    start=True, stop=False, skip_group_check=True,
                )

            sbA = sbpool.tile([128, HALF], BF16, tag="sbA", name="sbA")
            sbB = sbpool.tile([128, HALF], BF16, tag="sbB", name="sbB")

            def SB(j):
                t = sbA if j < 4 else sbB
                return t[:, (j % 4) * DVS:(j % 4 + 1) * DVS]

            n_total = n_chunks * reps

            def chunk_qh(ci):
                c = ci % n_chunks
                return divmod(c, 4)

            # per-chunk tile handles, filled as the pipeline advances
            kt_tiles = {}    # quad -> sbuf tile
            ktl_tiles = {}   # quad -> sbuf tile (alpha*kn rows)
            vb_tiles = {}    # group (8 chunks) -> sbuf tile
            rout_tiles = {}  # quad -> fp32 out staging
            pt_tiles = {}    # ci -> PT bf16
            ct_tiles = {}    # ci -> scaled cross-gram bf16
            psG_tiles = {}   # ci -> psum with G-base (+cross +R later)
            u_tiles = {}     # ci -> U bf16

            def KT(ci, j):
                q, h = chunk_qh(ci)
                ktv = kt_tiles[ci // 4][:].rearrange("p (j c) -> p j c", j=NJ)
                return ktv[:, j, h * C:(h + 1) * C]

            def dma_in(ci):
                """Issue input DMAs needed ahead of chunk ci."""
                quad = ci // 4
                q, _ = chunk_qh(ci)
                if ci % 4 == 0 and quad not in kt_tiles:
                    t = ktpool.tile([128, NJ * 4 * C], BF16, tag="kt", name="ktt_sb")
                    nc.sync.dma_start(
                        t[:].rearrange("p (j c) -> p j c", j=NJ),
                        ktt[q].rearrange("j p c -> p j c"),
                    )
                    kt_tiles[quad] = t
                if ci % 4 == 1 and quad + 1 not in kt_tiles and ci // 4 + 1 < (n_total + 3) // 4:
                    nq = chunk_qh(ci + 4)[0] if ci + 4 < n_total else None
                    if nq is not None:
                        t = ktpool.tile([128, NJ * 4 * C], BF16, tag="kt", name="ktt_sb")
                        nc.sync.dma_start(
                            t[:].rearrange("p (j c) -> p j c", j=NJ),
                            ktt[nq].rearrange("j p c -> p j c"),
                        )
                        kt_tiles[quad + 1] = t
                if ci % 4 == 0 and quad not in ktl_tiles:
                    t = ktlpool.tile([C, 4 * DK], BF16, tag="ktl", name="ktl_sb")
                    nc.sync.dma_start(
                        t[:].rearrange("p (i d) -> p i d", i=4),
                        ktl[4 * q:4 * (q + 1)].rearrange("i p d -> p i d"),
                    )
                    ktl_tiles[quad] = t
                if ci % 4 == 2 and ci + 4 < n_total and quad + 1 not in ktl_tiles:
                    nq = chunk_qh(ci + 4)[0]
                    t = ktlpool.tile([C, 4 * DK], BF16, tag="ktl", name="ktl_sb")
                    nc.sync.dma_start(
                        t[:].rearrange("p (i d) -> p i d", i=4),
                        ktl[4 * nq:4 * (nq + 1)].rearrange("i p d -> p i d"),
                    )
                    ktl_tiles[quad + 1] = t
                grp = ci // 8
                if ci % 8 == 0 and grp not in vb_tiles:
                    c0 = (ci % n_chunks)
                    t = vbpool.tile([C, 8 * DVS], BF16, tag="vb", name="vb_sb")
                    nc.sync.dma_start(
                        t[:].rearrange("p (i d) -> p i d", i=8),
                        vb[c0:c0 + 8].rearrange("i p d -> p i d"),
                    )
                    vb_tiles[grp] = t
                if ci % 8 == 4 and ci + 4 < n_total:
                    c0 = (ci + 4) % n_chunks
                    t = vbpool.tile([C, 8 * DVS], BF16, tag="vb", name="vb_sb")
                    nc.sync.dma_start(
                        t[:].rearrange("p (i d) -> p i d", i=8),
                        vb[c0:c0 + 8].rearrange("i p d -> p i d"),
                    )
                    vb_tiles[grp + 1] = t

            def prep_chunk(ci):
                """Everything for chunk ci that doesn't need u_{ci-1}:
                Gram(ci)+PT(ci), cross-gram source for ct(ci+1), state
                refresh + G-base(ci)."""
                if ci >= n_total:
                    return
                q, h = chunk_qh(ci)
                # Gram(ci) and (K_ci K_{ci+1}^T) in one matmul group when the
                # two chunks live in the same kt tile; else two groups.
                psA = psAp.tile([C, 2 * C], FP32, name="psA_t")
                have_next = ci + 1 < n_total
                if have_next and use_cross and h < 3:
                    ktv = kt_tiles[ci // 4][:].rearrange("p (j c) -> p j c", j=NJ)
                    for j in range(NJ):
                        nc.tensor.matmul(
                            psA[:], KT(ci, j), ktv[:, j, h * C:(h + 2) * C],
                            start=(j == 0), stop=(j == NJ - 1),
                        )
                else:
                    for j in range(NJ):
                        nc.tensor.matmul(
                            psA[:, 0:C], KT(ci, j), KT(ci, j),
                            start=(j == 0), stop=(j == NJ - 1),
                            skip_group_check=True,
                        )
                    if have_next and use_cross:
                        for j in range(NJ):
                            nc.tensor.matmul(
                                psA[:, C:2 * C], KT(ci, j), KT(ci + 1, j),
                                start=(j == 0), stop=(j == NJ - 1),
                                skip_group_check=True,
                            )
                pt = wk.tile([C, C], BF16, tag="pt", name="pt_t")
                nc.vector.tensor_mul(pt[:], psA[:, 0:C], mu[:])
                pt_tiles[ci] = pt
                if have_next and use_cross:
                    ct = wk.tile([C, C], BF16, tag="ct", name="ct_t")
                    nc.scalar.activation(
                        ct[:], psA[:, C:2 * C], mybir.ActivationFunctionType.Copy,
                        scale=ALPHA,
                    )
                    ct_tiles[ci + 1] = ct
                # refresh state snapshot (S after chunk ci-2) and G-base
                nc.scalar.activation(
                    sbA[:], psS0[:], mybir.ActivationFunctionType.Copy,
                )
                nc.scalar.activation(
                    sbB[:], psS1[:], mybir.ActivationFunctionType.Copy,
                )
                psG = psGp.tile([C, DVS], FP32, name="psG_t")
                for j in range(NJ):
                    nc.tensor.matmul(
                        psG[:], KT(ci, j), SB(j),
                        start=(j == 0), stop=False, skip_group_check=True,
                    )
                psG_tiles[ci] = psG

            # prologue: inputs + prep for chunk 0
            dma_in(0)
            prep_chunk(0)

            for ci in range(n_total):
                q, h = chunk_qh(ci)
                dma_in(ci + 1)

                psG = psG_tiles.pop(ci)
                pt = pt_tiles.pop(ci)
                # cross-term: psG += alpha*K_ci K_{ci-1}^T @ U_{ci-1}
                if ci > 0 and use_cross:
                    nc.tensor.matmul(
                        psG[:], ct_tiles.pop(ci), u_tiles.pop(ci - 1),
                        start=False, stop=False, skip_group_check=True,
                    )
                if not use_cross:
                    u_tiles.pop(ci - 1, None)

                # solve: X0 = V - G ; U = X0 - PT.T@X0
                vb_t = vb_tiles[ci // 8][:, (ci % 8) * DVS:(ci % 8 + 1) * DVS]
                x0 = wk.tile([C, DVS], BF16, tag="x0", name="x0_t")
                nc.vector.tensor_sub(x0[:], vb_t, psG[:])
                psX0 = psXp.tile([C, DVS], FP32, tag="psx", name="psX_t")
                nc.tensor.matmul(psX0[:], pt[:], x0[:], start=True, stop=True)
                u = wk.tile([C, DVS], BF16, tag="u", name="u_t")
                nc.vector.tensor_sub(u[:], x0[:], psX0[:])
                u_tiles[ci] = u

                # cross variant: prep ci+1 now - its refresh must READ psS
                # (state after ci-1) BEFORE this chunk's update writes it.
                if use_cross:
                    prep_chunk(ci + 1)

                # state update
                ktl_t = ktl_tiles[ci // 4][:, h * DK:(h + 1) * DK]
                for j in range(NJ):
                    js = slice((j % 4) * DVS, (j % 4 + 1) * DVS)
                    nc.tensor.matmul(
                        psS_half[j // 4][:, js], ktl_t[:, j * 128:(j + 1) * 128], u[:],
                        start=False, stop=(ci == n_total - 1), skip_group_check=True,
                    )

                # outputs: R = G(+cross) + PT.T @ U
                nc.tensor.matmul(
                    psG[:], pt[:], u[:],
                    start=False, stop=True, skip_group_check=True,
                )
                if h == 0:
                    rout_tiles[ci // 4] = routpool.tile(
                        [C, 4 * DVS], FP32, tag="routq", name="rout_q"
                    )
                rout_q = rout_tiles[ci // 4]
                nc.scalar.activation(
                    rout_q[:, h * DVS:(h + 1) * DVS], psG[:],
                    mybir.ActivationFunctionType.Copy, scale=1.0 / ALPHA,
                )
                if h == 3:
                    c_base = (ci % n_chunks) - 3
                    nc.gpsimd.dma_start(
                        r[c_base:c_base + 4].rearrange("i p d -> p i d"),
                        rout_tiles.pop(ci // 4)[:].rearrange("p (i d) -> p i d", i=4),
                    )

                if not use_cross:
                    prep_chunk(ci + 1)

                # release consumed input tiles
                if h == 3:
                    kt_tiles.pop(ci // 4 - 1, None)
                    ktl_tiles.pop(ci // 4 - 1, None)
                if ci % 8 == 7:
                    vb_tiles.pop(ci // 8 - 1, None)

    return nc


_nc_cache = {}


def _get_nc(zero_init: bool):
    if zero_init not in _nc_cache:
        _nc_cache[zero_init] = build_kernel(zero_init=zero_init)
    return _nc_cache[zero_init]


def _prep_inputs(k: np.ndarray, v: np.ndarray, W0: np.ndarray):
    k = np.asarray(k, np.float32)
    v = np.asarray(v, np.float32)
    W0 = np.asarray(W0, np.float32)
    kn = k / np.maximum(np.linalg.norm(k, axis=-1, keepdims=True), EPS)
    knb = kn.astype(ml_dtypes.bfloat16)
    # kT tiles: [NPAIR, NJ, 128, 2C] from knb.T [DK, T]
    ktt = np.ascontiguousarray(
        knb.T.reshape(NJ, 128, NQUAD, 4 * C).transpose(2, 0, 1, 3)
    )
    ktl = (ALPHA * knb.astype(np.float32)).astype(ml_dtypes.bfloat16) \
        .reshape(NCH, C, DK)
    s0_full = (ALPHA * W0.mean(axis=0)).astype(np.float32)  # [DK, DV]
    shared = {"ktt": ktt, "ktl": np.ascontiguousarray(ktl)}
    per_core = []
    for i in range(8):
        cs = slice(i * DVS, (i + 1) * DVS)
        vb = v[:, cs].astype(ml_dtypes.bfloat16).reshape(NCH, C, DVS)
        s0 = np.ascontiguousarray(s0_full[:, cs]).reshape(NJ, 128, DVS)
        per_core.append({**shared, "vb": np.ascontiguousarray(vb), "s0": s0})
    return per_core


def run(k, v, W0, trace=False, **kwargs):
    nc = _get_nc(zero_init=not np.any(np.asarray(W0)))
    in_maps = _prep_inputs(k, v, W0)
    res = run_bass_kernel_spmd(nc, in_maps, core_ids=list(range(8)),
                               trace=trace, **kwargs)
    out = np.concatenate(
        [res.results[i]["r"].reshape(T, DVS) for i in range(8)], axis=1
    )
    return out, res


def kernel(k, v, W0):
    out, _ = run(k, v, W0)
    return out.astype(np.float32)

